# revision 14
# baseline (speedup 1.0000x reference)
"""DeepAR sampling kernel for Trainium2 (8 NeuronCores, Bass/Tile).

Strategy:
 - Pure data-parallel sharding of the 8192 Monte-Carlo samples: 1024/core.
 - The GRU encoder (336 steps, batch 1) runs replicated on every core.
 - The sampling decoder (48 steps, autoregressive) runs fully on device,
   including the StudentT draw: jax.random.t's Marsaglia-Tsang rejection
   sampler is decomposed into df-independent random streams (normals,
   uniforms and their logs -- precomputed on host from the fixed seed via
   the exact jax threefry key chain) plus an on-device, df-dependent
   accept/reject SELECTION, which is just fp32 arithmetic + Ln/Exp.
 - No cross-core communication; the host gathers the 8 sample shards.
"""
import os
import sys
import subprocess
import tempfile
import hashlib
import numpy as np

# ---------------- problem constants (hardcoded per contract) ---------------
S_TOTAL = 8192
N_CORES = 8
SC = S_TOTAL // N_CORES         # samples per core
H = 512
G3 = 3 * H                      # 1536 stacked gates
HC = H // 128                   # h chunks (4)
MC = G3 // 128                  # gate-row chunks (12)
T_HIST = 336
P_LEN = 48
EMB = 40
NFEAT = 1 + EMB                 # GRU input features
R_ROUNDS = 8                    # precomputed rejection rounds
J_DRAWS = 4                     # precomputed inner (v<=0) redraws
SEED = 42
MIN_SCALE = 1e-5
ONE_THIRD = float(np.float32(1.0 / 3.0))

# =====================================================================
# Host-side stream extraction (subprocess on CPU jax so threefry/normal/
# uniform/log match the XLA-CPU reference bit-for-bit).
# =====================================================================
_STREAM_SRC = r'''
import numpy as np, sys
import jax, jax.numpy as jnp
from jax._src import random as jr

R, J, K, S, SEED, OUT = %d, %d, %d, %d, %d, %r
base = jax.random.key(SEED)

def per_step(k):
    tkey = jax.random.fold_in(base, k)
    key_n, key_g = jr._split(tkey)
    n = jr.normal(key_n, (S,), dtype=jnp.float32)
    gkeys = jax.vmap(jr._split, in_axes=(0, None))(
        tkey[None].at[0].set(key_g) if False else key_g.reshape(1), S
    ).reshape(S)

    def per_elem(ekey):
        key, _sub = jr._split(ekey)
        xs, us = [], []
        cur = key
        for _r in range(R):
            cur, x_key, U_key = jr._split(cur, 3)
            xj, xcur = [], x_key
            for _j in range(J):
                xcur, sub = jr._split(xcur)
                xj.append(jr.normal(sub, (), dtype=jnp.float32))
            xs.append(jnp.stack(xj))
            us.append(jr.uniform(U_key, (), dtype=jnp.float32))
        return jnp.stack(xs), jnp.stack(us)

    x, u = jax.vmap(per_elem)(gkeys)
    return n, jnp.transpose(x, (1, 2, 0)), jnp.transpose(u, (1, 0))

f = jax.jit(per_step)
ns, xs, us = [], [], []
for k in range(K):
    n, x, u = f(k)
    ns.append(np.asarray(n)); xs.append(np.asarray(x)); us.append(np.asarray(u))
np.savez(OUT, n=np.stack(ns), x=np.stack(xs), u=np.stack(us))
print("streams done")
'''


def _get_streams():
    key = f"deepar_streams_v3_{SEED}_{P_LEN}_{S_TOTAL}_{R_ROUNDS}_{J_DRAWS}"
    cache = os.path.join(tempfile.gettempdir(),
                         key + hashlib.md5(key.encode()).hexdigest()[:8] + ".npz")
    if not os.path.exists(cache):
        src = _STREAM_SRC % (R_ROUNDS, J_DRAWS, P_LEN, S_TOTAL, SEED, cache)
        env = dict(os.environ)
        env["JAX_PLATFORMS"] = "cpu"
        env["TRN_TERMINAL_POOL_IPS"] = ""          # skip axon boot
        env["PYTHONPATH"] = ":".join(p for p in sys.path if p)
        r = subprocess.run([sys.executable, "-c", src], env=env,
                           capture_output=True, text=True, timeout=1800)
        if r.returncode != 0 or not os.path.exists(cache):
            raise RuntimeError(f"stream extraction failed:\n{r.stdout}\n{r.stderr}")
    z = np.load(cache)
    return {"n": z["n"], "x": z["x"], "u": z["u"]}


# =====================================================================
# Bass kernel builder
# =====================================================================
def build_nc(T=T_HIST, P=P_LEN, enc_unroll=4, dec_unroll=2, debug=False,
             t_run=None, p_run=None):
    import concourse.bass as bass
    import concourse.bacc as bacc
    import concourse.tile as tile
    import concourse.mybir as mybir

    dt = mybir.dt
    f32 = dt.float32
    ALU = mybir.AluOpType
    ACTF = mybir.ActivationFunctionType

    nc = bacc.Bacc("TRN2", target_bir_lowering=False, debug=debug,
                   num_devices=N_CORES)

    def din(name, shape):
        return nc.dram_tensor(name, list(shape), f32, kind="ExternalInput")

    def dout(name, shape):
        return nc.dram_tensor(name, list(shape), f32, kind="ExternalOutput")

    # ---- dram params ----
    enc_x_d = din("enc_x", [1, T])
    xmT_d = din("xmT", [4, T])
    ymT_d = din("ymT", [4, P])
    WembT_d = din("WembT", [4, EMB])
    bemb_d = din("bemb", [EMB, 1])
    wih_d = din("wih", [NFEAT, G3])            # W_ih.T
    whh_d = din("whh", [128, HC, G3])          # W_hh.T chunked [p,kc,col]
    wproj_d = din("wproj", [128, HC, 3])       # [W_df|W_loc|W_sc].T chunked
    bproj_d = din("bproj", [3, 1])
    wcol0_d = din("wcol0", [128, MC])          # W_ih[:,0] chunk-major
    bfold_d = din("bfold", [128, MC])          # b_ih+b_hh (rz rows) / b_ih (n rows)
    bhhn_d = din("bhhn", [128, HC])            # b_hh n-rows chunk-major
    eye_d = din("eye", [128, 128])
    sel8_d = din("sel8", [8, 8 * 128])         # block-broadcast stationaries
    iota_d = din("iotaR", [128, 8, R_ROUNDS])  # r index along innermost
    xdr_d = din("xdr", [P, 128, J_DRAWS, 8, R_ROUNDS])
    udr_d = din("udr", [P, 128, 8, R_ROUNDS])
    ludr_d = din("ludr", [P, 128, 8, R_ROUNDS])
    ndr_d = din("ndr", [P, 128, 8])

    outs_d = dout("outs", [P, 8, 128])
    dfout_d = dout("dfout", [P, 128, 8])
    htout_d = dout("htout", [128, HC])

    with tile.TileContext(nc) as tc:
      with tc.tile_pool(name="const", bufs=1) as cp:
        # ---- persistent tiles ----
        enc_x = cp.tile([1, T], f32, tag="enc_x")
        encT = cp.tile([NFEAT, T], f32, tag="encT")
        emby = cp.tile([EMB, P], f32, tag="emby")
        wih = cp.tile([NFEAT, G3], f32, tag="wih")
        whh = cp.tile([128, HC, G3], f32, tag="whh")
        wproj = cp.tile([128, HC, 3], f32, tag="wproj")
        bproj = cp.tile([3, 1], f32, tag="bproj")
        wcol0 = cp.tile([128, MC], f32, tag="wcol0")
        bfold = cp.tile([128, MC], f32, tag="bfold")
        bhhn = cp.tile([128, HC], f32, tag="bhhn")
        bemb = cp.tile([EMB, 1], f32, tag="bemb")
        WembT = cp.tile([4, EMB], f32, tag="WembT")
        xmT = cp.tile([4, T], f32, tag="xmT")
        ymT = cp.tile([4, P], f32, tag="ymT")
        eye = cp.tile([128, 128], f32, tag="eye")
        sel8 = cp.tile([8, 8 * 128], f32, tag="sel8")
        iotaR = cp.tile([128, 8, R_ROUNDS], f32, tag="iotaR")
        ones1 = cp.tile([1, 128], f32, tag="ones1")
        wih2 = cp.tile([EMB, G3], f32, tag="wih2")
        gienc = cp.tile([128, MC, T], f32, tag="gienc")
        gidec = cp.tile([128, MC, P], f32, tag="gidec")
        h_e = [cp.tile([128, HC], f32, tag=f"h_e{i}", name=f"h_e{i}") for i in range(2)]
        hT = [cp.tile([128, HC, SC], f32, tag=f"hT{i}", name=f"hT{i}") for i in range(2)]
        tbc = [cp.tile([128, SC], f32, tag=f"tbc{i}", name=f"tbc{i}") for i in range(2)]
        r_sb = cp.tile([128, HC, SC], f32, tag="r_sb")
        z_sb = cp.tile([128, HC, SC], f32, tag="z_sb")
        dlsb = cp.tile([128, 24], f32, tag="dlsb")
        scl_col = cp.tile([128, 1], f32, tag="scl")
        scale_sb = cp.tile([1, 1], f32, tag="scale")
        inv_sb = cp.tile([1, 1], f32, tag="inv")
        tmp1 = cp.tile([1, 1], f32, tag="tmp1")
        tmp2 = cp.tile([1, 1], f32, tag="tmp2")

        nc.sync.dma_start(enc_x[:], enc_x_d.ap())
        nc.sync.dma_start(xmT[:], xmT_d.ap())
        nc.sync.dma_start(ymT[:], ymT_d.ap())
        nc.sync.dma_start(WembT[:], WembT_d.ap())
        nc.sync.dma_start(bemb[:], bemb_d.ap())
        nc.sync.dma_start(wih[:], wih_d.ap())
        nc.sync.dma_start(wih2[:], wih_d.ap()[0:EMB, :])
        nc.sync.dma_start(whh[:], whh_d.ap())
        nc.sync.dma_start(wproj[:], wproj_d.ap())
        nc.sync.dma_start(bproj[:], bproj_d.ap())
        nc.sync.dma_start(wcol0[:], wcol0_d.ap())
        nc.sync.dma_start(bfold[:], bfold_d.ap())
        nc.sync.dma_start(bhhn[:], bhhn_d.ap())
        nc.sync.dma_start(eye[:], eye_d.ap())
        nc.sync.dma_start(sel8[:], sel8_d.ap())
        nc.sync.dma_start(iotaR[:], iota_d.ap())
        nc.vector.memset(ones1[:], 1.0)

        with tc.tile_pool(name="pre_ps", bufs=2, space="PSUM") as pps:
            # scale = clip(mean|x|, 1e-5); inv = 1/scale (Newton-refined)
            absx = cp.tile([1, T], f32, tag="absx")
            nc.scalar.activation(absx[:], enc_x[:], ACTF.Abs)
            ssum = cp.tile([1, 1], f32, tag="ssum")
            nc.vector.tensor_reduce(ssum[:], absx[:], mybir.AxisListType.X,
                                    ALU.add)
            nc.vector.tensor_scalar(scale_sb[:], ssum[:], float(np.float32(1.0 / T)),
                                    float(np.float32(MIN_SCALE)), ALU.mult, ALU.max)
            nc.vector.reciprocal(inv_sb[:], scale_sb[:])
            # Newton: inv = inv*(2 - scale*inv)
            nc.vector.tensor_tensor(tmp1[:], scale_sb[:], inv_sb[:], ALU.mult)
            nc.vector.tensor_scalar(tmp2[:], tmp1[:], -1.0, 2.0, ALU.mult, ALU.add)
            nc.vector.tensor_tensor(inv_sb[:], inv_sb[:], tmp2[:], ALU.mult)

            # encT rows 0..39 = tfeT = Wemb @ xmT + bemb ; row 40 = seq
            seqsc = cp.tile([1, T], f32, tag="seqsc")
            nc.vector.tensor_scalar(seqsc[:], enc_x[:], inv_sb[0:1, 0:1],
                                    None, ALU.mult)
            nc.sync.dma_start(encT[EMB:NFEAT, :], seqsc[:])
            tfe_ps = pps.tile([EMB, T], f32, tag="tfe")
            nc.tensor.matmul(tfe_ps[:], WembT[:], xmT[:], start=True, stop=True)
            nc.scalar.activation(encT[0:EMB, :], tfe_ps[:], ACTF.Identity,
                                 bias=bemb[:])
            # emby = Wemb @ ymT + bemb
            emb_ps = pps.tile([EMB, P], f32, tag="emby")
            nc.tensor.matmul(emb_ps[:], WembT[:], ymT[:], start=True, stop=True)
            nc.scalar.activation(emby[:], emb_ps[:], ACTF.Identity, bias=bemb[:])

        with tc.tile_pool(name="gi_ps", bufs=2, space="PSUM") as gps:
            # gienc[:, mc, :] = (W_ih.T chunk).T @ encT + bfold
            for mc in range(MC):
                ps = gps.tile([128, T], f32, tag="gienc_ps")
                nc.tensor.matmul(ps[:], wih[:, mc * 128:(mc + 1) * 128], encT[:],
                                 start=True, stop=True)
                nc.scalar.activation(gienc[:, mc, :], ps[:], ACTF.Identity,
                                     bias=bfold[:, mc:mc + 1])
            # gidec[:, mc, :] = (W_ih.T rows 1..41 chunk).T @ emby + bfold
            for mc in range(MC):
                ps = gps.tile([128, P], f32, tag="gidec_ps")
                nc.tensor.matmul(ps[:], wih2[:, mc * 128:(mc + 1) * 128],
                                 emby[:], start=True, stop=True)
                nc.scalar.activation(gidec[:, mc, :], ps[:], ACTF.Identity,
                                     bias=bfold[:, mc:mc + 1])

        nc.vector.memset(h_e[0][:], 0.0)
        nc.vector.memset(h_e[1][:], 0.0)

        # ================= encoder loop =================
        with (
            tc.tile_pool(name="enc_sb", bufs=2) as esb,
            tc.tile_pool(name="enc_ps", bufs=2, space="PSUM") as eps,
        ):
            def enc_step(t, cur, nxt):
                gie = esb.tile([128, MC], f32, tag="gie")
                nc.vector.tensor_copy(gie[:], gienc[:, :, t])
                # h-stationary matvec: gh rows [1,512] per gate, psum-accum
                # over kc, then transpose back to column chunks.
                grow = esb.tile([1, G3], f32, tag="grow")
                for g in range(3):
                    pr = eps.tile([1, 512], f32, tag="encrow")
                    for kc in range(HC):
                        nc.tensor.matmul(
                            pr[:], cur[:, kc:kc + 1],
                            whh[:, kc, g * 512:(g + 1) * 512],
                            start=(kc == 0), stop=(kc == HC - 1))
                    nc.vector.tensor_copy(grow[:, g * 512:(g + 1) * 512], pr[:])
                ps = eps.tile([128, MC], f32, tag="encps")
                for mc in range(MC):
                    nc.tensor.transpose(ps[:, mc:mc + 1],
                                        grow[:, mc * 128:(mc + 1) * 128],
                                        eye[0:1, 0:1])
                ps_sb = esb.tile([128, MC], f32, tag="ps_sb")
                nc.scalar.activation(ps_sb[:], ps[:], ACTF.Copy)
                ps = ps_sb
                rzp = esb.tile([128, 8], f32, tag="rzp")
                nc.vector.tensor_tensor(rzp[:], ps[:, 0:8], gie[:, 0:8], ALU.add)
                rz = esb.tile([128, 8], f32, tag="rz")
                nc.scalar.activation(rz[:], rzp[:], ACTF.Sigmoid)
                hnb = esb.tile([128, HC], f32, tag="hnb")
                nc.vector.tensor_tensor(hnb[:], ps[:, 8:12], bhhn[:], ALU.add)
                t1 = esb.tile([128, HC], f32, tag="t1")
                nc.vector.tensor_tensor(t1[:], rz[:, 0:4], hnb[:], ALU.mult)
                nc.vector.tensor_tensor(t1[:], t1[:], gie[:, 8:12], ALU.add)
                nn = esb.tile([128, HC], f32, tag="nn")
                nc.scalar.activation(nn[:], t1[:], ACTF.Tanh)
                # h' = (1-z)*n + z*h  (exact XLA op order)
                omz = esb.tile([128, HC], f32, tag="omz")
                nc.vector.tensor_scalar(omz[:], rz[:, 4:8], -1.0, 1.0,
                                        ALU.mult, ALU.add)
                nc.vector.tensor_tensor(omz[:], omz[:], nn[:], ALU.mult)
                zh = esb.tile([128, HC], f32, tag="zh")
                nc.vector.tensor_tensor(zh[:], rz[:, 4:8], cur[:], ALU.mult)
                nc.vector.tensor_tensor(nxt[:], omz[:], zh[:], ALU.add)

            def enc_body(t0):
                for q in range(enc_unroll):
                    enc_step(t0 + q, h_e[q % 2], h_e[(q + 1) % 2])

            T_RUN = T if t_run is None else t_run
            if enc_unroll == 0:
                for t0 in range(0, T_RUN, 2):
                    for q in range(2):
                        enc_step(t0 + q, h_e[q % 2], h_e[(q + 1) % 2])
            else:
                with tc.For_i(0, T_RUN, enc_unroll) as t0:
                    enc_body(t0)

        h_fin = h_e[0]
        nc.sync.dma_start(htout_d.ap(), h_fin[:])

        # ============== decoder init ==============
        with tc.tile_pool(name="dini_ps", bufs=2, space="PSUM") as ips:
            for c in range(HC):
                nc.vector.memset(hT[0][:, c, :], 0.0)
                nc.vector.tensor_scalar(hT[0][:, c, :], hT[0][:, c, :],
                                        h_fin[:, c:c + 1], None, ALU.add)
            # tgt0 = x[-1] broadcast ; scl_col = scale broadcast
            t0ps = ips.tile([128, 1], f32, tag="t0ps")
            nc.tensor.matmul(t0ps[:], ones1[:], enc_x[0:1, T - 1:T],
                             start=True, stop=True)
            nc.vector.memset(tbc[0][:], 0.0)
            nc.vector.tensor_scalar(tbc[0][:], tbc[0][:], t0ps[:, 0:1], None,
                                    ALU.add)
            sps = ips.tile([128, 1], f32, tag="sps")
            nc.tensor.matmul(sps[:], ones1[:], scale_sb[:], start=True, stop=True)
            nc.scalar.activation(scl_col[:], sps[:], ACTF.Copy)

        # ============== decoder loop ==============
        NH = SC // 512
        with (
            tc.tile_pool(name="dec_sb", bufs=3) as dsb,
            tc.tile_pool(name="gam_sb", bufs=2) as gsb,
            tc.tile_pool(name="str_sb", bufs=2) as ssb,
            tc.tile_pool(name="g_ps", bufs=3, space="PSUM") as gps2,
            tc.tile_pool(name="p_ps", bufs=1, space="PSUM") as p_ps,
            tc.tile_pool(name="t_ps", bufs=2, space="PSUM") as t_ps,
        ):
            def dec_step(k, cur, nxt, tb_cur, tb_nxt):
                # stream tiles for this step
                xst = ssb.tile([128, J_DRAWS, 8, R_ROUNDS], f32, tag="xst")
                ust = ssb.tile([128, 8, R_ROUNDS], f32, tag="ust")
                lust = ssb.tile([128, 8, R_ROUNDS], f32, tag="lust")
                nst = ssb.tile([128, 8], f32, tag="nst")
                nc.sync.dma_start(xst[:], xdr_d.ap()[k])
                nc.sync.dma_start(ust[:], udr_d.ap()[k])
                nc.sync.dma_start(lust[:], ludr_d.ap()[k])
                nc.sync.dma_start(nst[:], ndr_d.ap()[k])
                gik = dsb.tile([128, MC], f32, tag="gik")
                nc.vector.tensor_copy(gik[:], gidec[:, :, k])

                # ---- gates ----
                for nh in range(NH):
                    nsl = slice(nh * 512, (nh + 1) * 512)
                    for mc in range(MC):
                        ps = gps2.tile([128, 512], f32, tag="gps")
                        for kc in range(HC):
                            nc.tensor.matmul(
                                ps[:], whh[:, kc, mc * 128:(mc + 1) * 128],
                                cur[:, kc, nsl],
                                start=(kc == 0), stop=(kc == HC - 1))
                        if mc < 8:
                            # r/z: pre = tbc*wcol0[mc] + psum ; act(+GiDec bias)
                            pre = dsb.tile([128, 512], f32, tag="pre")
                            nc.vector.scalar_tensor_tensor(
                                pre[:], tb_cur[:, nsl], wcol0[:, mc:mc + 1],
                                ps[:], ALU.mult, ALU.add)
                            dst = r_sb if mc < 4 else z_sb
                            nc.scalar.activation(dst[:, mc % 4, nsl], pre[:],
                                                 ACTF.Sigmoid,
                                                 bias=gik[:, mc:mc + 1])
                        else:
                            c = mc - 8
                            hnb = dsb.tile([128, 512], f32, tag="hnb")
                            nc.scalar.activation(hnb[:], ps[:], ACTF.Identity,
                                                 bias=bhhn[:, c:c + 1])
                            t1 = dsb.tile([128, 512], f32, tag="t1d")
                            nc.vector.tensor_tensor(t1[:], r_sb[:, c, nsl],
                                                    hnb[:], ALU.mult)
                            nc.vector.scalar_tensor_tensor(
                                t1[:], tb_cur[:, nsl], wcol0[:, mc:mc + 1],
                                t1[:], ALU.mult, ALU.add)
                            nn = dsb.tile([128, 512], f32, tag="nnd")
                            nc.scalar.activation(nn[:], t1[:], ACTF.Tanh,
                                                 bias=gik[:, mc:mc + 1])
                            # h' = (1-z)*n + z*h
                            omz = dsb.tile([128, 512], f32, tag="omzd")
                            nc.vector.tensor_scalar(omz[:], z_sb[:, c, nsl],
                                                    -1.0, 1.0, ALU.mult, ALU.add)
                            nc.vector.tensor_tensor(omz[:], omz[:], nn[:],
                                                    ALU.mult)
                            zh = dsb.tile([128, 512], f32, tag="zhd")
                            nc.vector.tensor_tensor(zh[:], z_sb[:, c, nsl],
                                                    cur[:, c, nsl], ALU.mult)
                            nc.vector.tensor_tensor(nxt[:, c, nsl], omz[:],
                                                    zh[:], ALU.add)

                # ---- projections on h_new ----
                for nh in range(NH):
                    nsl = slice(nh * 512, (nh + 1) * 512)
                    pp = p_ps.tile([3, 512], f32, tag="pp")
                    for kc in range(HC):
                        nc.tensor.matmul(pp[:], wproj[:, kc, :],
                                         nxt[:, kc, nsl],
                                         start=(kc == 0), stop=(kc == HC - 1))
                    prj = dsb.tile([3, 512], f32, tag="prj")
                    nc.scalar.activation(prj[:], pp[:], ACTF.Identity,
                                         bias=bproj[:])
                    for j in range(4):
                        sj = nh * 4 + j
                        tp = t_ps.tile([128, 3], f32, tag="tpA")
                        nc.tensor.transpose(tp[:], prj[:, j * 128:(j + 1) * 128],
                                            eye[0:3, 0:3])
                        nc.scalar.activation(dlsb[:, 3 * sj:3 * sj + 3], tp[:],
                                             ACTF.Copy)

                df_raw = dlsb[:, 0:24:3]
                loc8 = dlsb[:, 1:24:3]
                sc_raw = dlsb[:, 2:24:3]

                # ---- df = 2 + softplus(df_raw) ;  sc = softplus(sc_raw) ----
                def softplus(dst, src):
                    ax = gsb.tile([128, 8], f32, tag="sp_a")
                    nc.scalar.activation(ax[:], src, ACTF.Abs)
                    ex = gsb.tile([128, 8], f32, tag="sp_e")
                    nc.scalar.activation(ex[:], ax[:], ACTF.Exp, scale=-1.0)
                    nc.vector.tensor_scalar(ex[:], ex[:], 1.0, None, ALU.add)
                    ln = gsb.tile([128, 8], f32, tag="sp_l")
                    nc.scalar.activation(ln[:], ex[:], ACTF.Ln)
                    rl = gsb.tile([128, 8], f32, tag="sp_r")
                    nc.vector.tensor_scalar(rl[:], src, 0.0, None, ALU.max)
                    nc.vector.tensor_tensor(dst, rl[:], ln[:], ALU.add)

                df8 = gsb.tile([128, 8], f32, tag="df8")
                softplus(df8[:], df_raw)
                nc.vector.tensor_scalar(df8[:], df8[:], 2.0, None, ALU.add)
                nc.sync.dma_start(dfout_d.ap()[k], df8[:])
                half8 = gsb.tile([128, 8], f32, tag="half8")
                nc.vector.tensor_scalar(half8[:], df8[:], 0.5, None, ALU.mult)
                d8 = gsb.tile([128, 8], f32, tag="d8")
                nc.vector.tensor_scalar(d8[:], half8[:], ONE_THIRD, None,
                                        ALU.subtract)
                lnd = gsb.tile([128, 8], f32, tag="lnd")
                nc.scalar.activation(lnd[:], d8[:], ACTF.Ln)
                c8 = gsb.tile([128, 8], f32, tag="c8")
                nc.scalar.activation(c8[:], lnd[:], ACTF.Exp, scale=-0.5)
                nc.vector.tensor_scalar(c8[:], c8[:], ONE_THIRD, None, ALU.mult)

                def rep8(name, src):
                    t = gsb.tile([128, 8, R_ROUNDS], f32, tag=name)
                    nc.vector.tensor_copy(t[:, :, 0], src)
                    nc.vector.tensor_copy(t[:, :, 1], t[:, :, 0])
                    nc.vector.tensor_copy(t[:, :, 2:4], t[:, :, 0:2])
                    nc.vector.tensor_copy(t[:, :, 4:8], t[:, :, 0:4])
                    return t

                crep = rep8("crep", c8[:])
                drep = rep8("drep", d8[:])

                # ---- inner-redraw selection: first j with v_j > 0 ----
                sel_a = gsb.tile([128, 8, R_ROUNDS], f32, tag="sel_a")
                sel_b = gsb.tile([128, 8, R_ROUNDS], f32, tag="sel_b")
                tv = gsb.tile([128, 8, R_ROUNDS], f32, tag="tv")
                tm = gsb.tile([128, 8, R_ROUNDS], dt.uint8, tag="tm")
                cur_sel, oth = sel_a, sel_b
                nc.vector.tensor_copy(cur_sel[:], xst[:, J_DRAWS - 1])
                for j in range(J_DRAWS - 2, -1, -1):
                    nc.vector.tensor_tensor(tv[:], xst[:, j], crep[:], ALU.mult)
                    nc.vector.tensor_scalar(tv[:], tv[:], 1.0, None, ALU.add)
                    nc.vector.tensor_scalar(tm[:], tv[:], 0.0, None, ALU.is_gt)
                    nc.vector.select(oth[:], tm[:], xst[:, j], cur_sel[:])
                    cur_sel, oth = oth, cur_sel
                selx = cur_sel
                vt = gsb.tile([128, 8, R_ROUNDS], f32, tag="vt")
                nc.vector.tensor_tensor(vt[:], selx[:], crep[:], ALU.mult)
                nc.vector.tensor_scalar(vt[:], vt[:], 1.0, None, ALU.add)

                Xt = gsb.tile([128, 8, R_ROUNDS], f32, tag="Xt")
                nc.vector.tensor_tensor(Xt[:], selx[:], selx[:], ALU.mult)
                xh = gsb.tile([128, 8, R_ROUNDS], f32, tag="xh")
                nc.vector.tensor_scalar(xh[:], Xt[:], 0.5, None, ALU.mult)
                Vt = gsb.tile([128, 8, R_ROUNDS], f32, tag="Vt")
                nc.vector.tensor_tensor(Vt[:], vt[:], vt[:], ALU.mult)
                nc.vector.tensor_tensor(Vt[:], Vt[:], vt[:], ALU.mult)
                x4 = gsb.tile([128, 8, R_ROUNDS], f32, tag="x4")
                nc.vector.tensor_tensor(x4[:], Xt[:], Xt[:], ALU.mult)
                # squeeze threshold 1 - 0.0331*x^4
                sq = gsb.tile([128, 8, R_ROUNDS], f32, tag="sq")
                nc.vector.tensor_scalar(sq[:], x4[:],
                                        float(np.float32(-0.0331)), 1.0,
                                        ALU.mult, ALU.add)
                sqr = gsb.tile([128, 8, R_ROUNDS], f32, tag="sqr")
                nc.vector.tensor_tensor(sqr[:], ust[:], sq[:], ALU.is_ge)
                lnv = gsb.tile([128, 8, R_ROUNDS], f32, tag="lnv")
                nc.scalar.activation(lnv[:], Vt[:], ACTF.Ln)
                omv = gsb.tile([128, 8, R_ROUNDS], f32, tag="omv")
                nc.vector.tensor_scalar(omv[:], Vt[:], -1.0, 1.0, ALU.mult,
                                        ALU.add)
                nc.vector.tensor_tensor(omv[:], omv[:], lnv[:], ALU.add)
                nc.vector.tensor_tensor(omv[:], drep[:], omv[:], ALU.mult)
                rhs = gsb.tile([128, 8, R_ROUNDS], f32, tag="rhs")
                nc.vector.tensor_tensor(rhs[:], xh[:], omv[:], ALU.add)
                mr = gsb.tile([128, 8, R_ROUNDS], f32, tag="mr")
                nc.vector.tensor_tensor(mr[:], lust[:], rhs[:], ALU.is_ge)
                rej = gsb.tile([128, 8, R_ROUNDS], f32, tag="rej")
                nc.vector.tensor_tensor(rej[:], sqr[:], mr[:], ALU.mult)
                score = gsb.tile([128, 8, R_ROUNDS], f32, tag="score")
                nc.vector.scalar_tensor_tensor(score[:], rej[:],
                                               float(R_ROUNDS), iotaR[:],
                                               ALU.mult, ALU.add)
                smin = gsb.tile([128, 8], f32, tag="smin")
                nc.vector.tensor_reduce(smin[:], score[:],
                                        mybir.AxisListType.X, ALU.min)
                sminr = rep8("sminr", smin[:])
                take = gsb.tile([128, 8, R_ROUNDS], f32, tag="take")
                nc.vector.tensor_tensor(take[:], score[:], sminr[:],
                                        ALU.is_equal)
                gv = gsb.tile([128, 8, R_ROUNDS], f32, tag="gv")
                nc.vector.tensor_tensor(gv[:], drep[:], Vt[:], ALU.mult)
                nc.vector.tensor_tensor(gv[:], take[:], gv[:], ALU.mult)
                g8 = gsb.tile([128, 8], f32, tag="g8")
                nc.vector.tensor_reduce(g8[:], gv[:], mybir.AxisListType.X,
                                        ALU.add)

                # eps = n * sqrt(half/g)  via exp(0.5*(ln half - ln g))
                lng = gsb.tile([128, 8], f32, tag="lng")
                nc.scalar.activation(lng[:], g8[:], ACTF.Ln)
                lnh = gsb.tile([128, 8], f32, tag="lnh")
                nc.scalar.activation(lnh[:], half8[:], ACTF.Ln)
                nc.vector.tensor_tensor(lnh[:], lnh[:], lng[:], ALU.subtract)
                qs = gsb.tile([128, 8], f32, tag="qs")
                nc.scalar.activation(qs[:], lnh[:], ACTF.Exp, scale=0.5)
                eps = gsb.tile([128, 8], f32, tag="eps")
                nc.vector.tensor_tensor(eps[:], nst[:], qs[:], ALU.mult)

                sc8 = gsb.tile([128, 8], f32, tag="sc8")
                softplus(sc8[:], sc_raw)
                smp = gsb.tile([128, 8], f32, tag="smp")
                nc.vector.tensor_tensor(smp[:], sc8[:], eps[:], ALU.mult)
                nc.vector.tensor_tensor(smp[:], loc8, smp[:], ALU.add)
                nc.vector.tensor_scalar(smp[:], smp[:], scl_col[:, 0:1], None,
                                        ALU.mult)

                # transpose samples to row layout, DMA out, broadcast to tbc
                tps = t_ps.tile([8, 128], f32, tag="tpA")
                nc.tensor.transpose(tps[:], smp[:], eye[:])
                srow = dsb.tile([8, 128], f32, tag="srow")
                nc.scalar.activation(srow[:], tps[:], ACTF.Copy)
                nc.sync.dma_start(outs_d.ap()[k], srow[:])
                for nh in range(NH):
                    bps = t_ps.tile([128, 512], f32, tag="tpB")
                    for q in range(4):
                        s0 = nh * 4 + q
                        nc.tensor.matmul(bps[:, q * 128:(q + 1) * 128],
                                         sel8[:, s0 * 128:(s0 + 1) * 128],
                                         srow[:], start=True, stop=True)
                    nc.scalar.activation(tb_nxt[:, nh * 512:(nh + 1) * 512],
                                         bps[:], ACTF.Copy)

            def dec_body(k0):
                for q in range(dec_unroll):
                    dec_step(k0 + q, hT[q % 2], hT[(q + 1) % 2],
                             tbc[q % 2], tbc[(q + 1) % 2])

            P_RUN = P if p_run is None else p_run
            if dec_unroll == 0:
                for k0 in range(0, P_RUN, 2):
                    for q in range(2):
                        dec_step(k0 + q, hT[q % 2], hT[(q + 1) % 2],
                                 tbc[q % 2], tbc[(q + 1) % 2])
            else:
                assert P % dec_unroll == 0 and dec_unroll % 2 == 0
                with tc.For_i(0, P_RUN, dec_unroll) as k0:
                    dec_body(k0)

    nc.compile()
    return nc


# =====================================================================
# Host-side input prep
# =====================================================================
def host_prep(inputs, streams, T=T_HIST, P=P_LEN):
    f32 = np.float32
    x = np.asarray(inputs["x"], f32)
    x_mark = np.asarray(inputs["x_mark"], f32)
    y_mark = np.asarray(inputs["y_mark"], f32)
    W_emb = np.asarray(inputs["W_emb"], f32)
    b_emb = np.asarray(inputs["b_emb"], f32)
    W_ih = np.asarray(inputs["W_ih"], f32)
    W_hh = np.asarray(inputs["W_hh"], f32)
    b_ih = np.asarray(inputs["b_ih"], f32)
    b_hh = np.asarray(inputs["b_hh"], f32)
    W_df = np.asarray(inputs["W_df"], f32)
    W_loc = np.asarray(inputs["W_loc"], f32)
    W_sc = np.asarray(inputs["W_sc"], f32)
    b_df = np.asarray(inputs["b_df"], f32)
    b_loc = np.asarray(inputs["b_loc"], f32)
    b_sc = np.asarray(inputs["b_sc"], f32)

    whhT = W_hh.T.copy()                       # [512, 1536]
    whh = whhT.reshape(HC, 128, G3).transpose(1, 0, 2).copy()
    wprojT = np.concatenate([W_df, W_loc, W_sc], 0).T.copy()   # [512, 3]
    wproj = wprojT.reshape(HC, 128, 3).transpose(1, 0, 2).copy()
    wcol0 = W_ih[:, 0].reshape(MC, 128).T.copy()
    bfold_v = np.concatenate([b_ih[:2 * H] + b_hh[:2 * H], b_ih[2 * H:]])
    bfold = bfold_v.reshape(MC, 128).T.copy()
    bhhn = b_hh[2 * H:].reshape(HC, 128).T.copy()
    bproj = np.stack([b_df[0], b_loc[0], b_sc[0]])[:, None].astype(f32)

    sel8 = np.zeros((8, 8 * 128), f32)
    for s0 in range(8):
        sel8[s0, s0 * 128:(s0 + 1) * 128] = 1.0
    iotaR = np.broadcast_to(
        np.arange(R_ROUNDS, dtype=f32)[None, None, :], (128, 8, R_ROUNDS)).copy()

    base = {
        "enc_x": x[0, :, 0][None, :].copy(),
        "xmT": x_mark[0].T.copy(),
        "ymT": y_mark[0].T.copy(),
        "WembT": W_emb.T.copy(),
        "bemb": b_emb[:, None].copy(),
        "wih": np.concatenate([W_ih.T[1:], W_ih.T[0:1]], 0).copy(),
        "whh": whh,
        "wproj": wproj,
        "bproj": bproj,
        "wcol0": wcol0,
        "bfold": bfold,
        "bhhn": bhhn,
        "eye": np.eye(128, dtype=f32),
        "sel8": sel8,
        "iotaR": iotaR,
    }

    xs, us, ns = streams["x"], streams["u"], streams["n"]   # [K,R,J,S],[K,R,S],[K,S]
    lus = np.log(us).astype(f32)
    in_maps = []
    for c in range(N_CORES):
        sl = slice(c * SC, (c + 1) * SC)
        xc = xs[:P, :, :, sl].reshape(P, R_ROUNDS, J_DRAWS, 8, 128)
        xc = xc.transpose(0, 4, 2, 3, 1).copy()       # [P,128,J,8,R]
        uc = us[:P, :, sl].reshape(P, R_ROUNDS, 8, 128).transpose(0, 3, 2, 1).copy()
        luc = lus[:P, :, sl].reshape(P, R_ROUNDS, 8, 128).transpose(0, 3, 2, 1).copy()
        nc_ = ns[:P, sl].reshape(P, 8, 128).transpose(0, 2, 1).copy()
        m = dict(base)
        m["xdr"] = xc.astype(f32)
        m["udr"] = uc.astype(f32)
        m["ludr"] = luc.astype(f32)
        m["ndr"] = nc_.astype(f32)
        in_maps.append(m)
    return in_maps


_NC_CACHE = {}
LAST_EXEC_NS = {}


def kernel(**inputs) -> np.ndarray:
    from concourse.bass_utils import run_bass_kernel_spmd

    streams = _get_streams()
    in_maps = host_prep(inputs, streams)
    if "nc" not in _NC_CACHE:
        _NC_CACHE["nc"] = build_nc()
    nc = _NC_CACHE["nc"]
    trace = bool(int(os.environ.get("DEEPAR_TRACE", "0")))
    res = run_bass_kernel_spmd(nc, in_maps, list(range(N_CORES)), trace=trace)
    LAST_EXEC_NS["ns"] = getattr(res, "exec_time_ns", None)
    LAST_EXEC_NS["res"] = res
    out = np.empty((S_TOTAL, P_LEN, 1), np.float32)
    for c in range(N_CORES):
        oc = np.asarray(res.results[c]["outs"])      # [P, 8, 128]
        out[c * SC:(c + 1) * SC, :, 0] = oc.reshape(P_LEN, SC).T
    return out
